# revision 48
# baseline (speedup 1.0000x reference)
"""Trainium2 Bass kernel for 16-head MultiHeadAttention (B=2, S=2048, D=1024).

Sharding: 8 cores = 2 (batch) x 4 (head groups of 4 heads).  Each core
computes, for its batch b and head group g:
  Q_g = x_q @ Wq[:, g] ; K_g, V_g likewise
  ctx_g = softmax(Q_g K_g^T / sqrt(64)) V_g            (4 heads)
  out_partial = ctx_g @ Wo[g, :]                        [2048, 1024]
Host sums the 4 partials per batch and adds bo.

v4 schedule notes (on top of the v3 layout):
  - trace-measured facts: fp16 matmul streams 1 row/cycle at 2.4 GHz
    (214ns per 512-row matmul); the row-split score matmul pairs DO
    overlap on HW (215ns/pair); exp is 1113ns per [128,1024] step, and
    128 exp steps = 142.5us is the hard ScalarE floor of this design
  - startup: DMA priority order wk, xk0 (split in f-halves so the K
    projection starts on the first half), wq, xq0, xk1 (halves), ...;
    warmup is 22 small 256-row matmuls that bridge the preamble->xk0
    window without overshooting, plus a few bridging units before Q0
  - fillers are all split into <=428ns units (2x512-row or 4x256-row
    matmuls); a deadline-ordered `urgent` queue (K/V/Q projections)
    pumps 2 units/step during the first LAG steps (no ctx matmuls yet,
    ~900ns PE slack per step) and 1 unit/step after; `lazy` units
    (out-proj matmul+store pairs, no deadline) only pump when urgent
    is empty
  - LAG (ctx lag behind scores) raised 16->20: V(t,j) filler deadlines
    move from step 16+4t+j to 20+4t+j, spreading the qt0 projection
    crunch into later windows that have slack; P pool sized LAG+2
  - tail: final out-proj PSUM->SBUF casts alternate DVE/gpsimd so the
    last 8 stores aren't serialized on one engine
"""

import os
import sys

sys.path.insert(0, "/opt/trn_rl_repo")

import numpy as np

import concourse.bass as bass
import concourse.tile as tile
from concourse import bacc, mybir
from concourse.bass_utils import run_bass_kernel_spmd

F32 = mybir.dt.float32
F16 = mybir.dt.float16
AF = mybir.ActivationFunctionType

D = 1024          # model dim
S = 2048          # sequence length (per batch)
HPC = 4           # heads per core
DK = 64           # head dim
HC = HPC * DK     # head cols per core = 256
FC = 8            # feature chunks of 128 (contraction for projections)
TT = 4            # token tiles of 512
KC = 16           # key chunks of 128
LAG = 20          # ctx-matmul lag behind score-matmuls (steps)

LAST_RESULTS = None  # BassKernelResults of the most recent run (for test.py)
_NC_CACHE = None


# move_matmul_waits_to_ldweights emits a standalone InstLdweights per
# matmul, which walrus's LDW optimization refuses; skip it and let
# generate_event_semaphores legalize multi-waits via event semaphores.
bacc.Bacc.move_matmul_waits_to_ldweights = lambda self: None
_Bacc = bacc.Bacc


def build_nc():
    # Bacc (not raw Bass): its compile() runs generate_event_semaphores,
    # which legalizes multi-semaphore waits down to the hardware limit.
    nc = _Bacc("TRN2", target_bir_lowering=False, debug=False)

    # xstart packs the startup-critical data in three DMAs sized to the
    # first-exp critical chain (each transfer has ~1.5-2us fixed latency
    # and early-kernel HBM bandwidth is well below steady-state, so the
    # two leading transfers carry only what the first score needs):
    #   slot0 [0:5120]      = wk(mt0) | xk0
    #   slot1 [5120:10240]  = wq(mt0) | xq0
    #   slot2 [10240:12288] = wk(mt1) | wq(mt1)
    xst = nc.dram_tensor("xstart", [128, 12288], F16, kind="ExternalInput")
    xq = nc.dram_tensor("xq_t", [128, TT, FC, 512], F16, kind="ExternalInput")
    xk = nc.dram_tensor("xk_t", [128, TT, FC, 512], F16, kind="ExternalInput")
    xv = nc.dram_tensor("xv_t", [128, TT, FC, 512], F16, kind="ExternalInput")
    wpk = nc.dram_tensor("wpack", [128, 2, FC * HC], F16, kind="ExternalInput")
    bq = nc.dram_tensor("bq2", [128, 2], F32, kind="ExternalInput")
    bk = nc.dram_tensor("bk2", [128, 2], F32, kind="ExternalInput")
    bv = nc.dram_tensor("bv_bc", [128, HC], F32, kind="ExternalInput")
    out_p = nc.dram_tensor("out_p", [D, S], F16, kind="ExternalOutput")

    with tile.TileContext(nc) as tc:
        _emit(tc, xst, xq, xk, xv, wpk, bq, bk, bv, out_p)
    nc.compile()
    return nc


def _emit(tc, xst, xq, xk, xv, wpk, bq, bk, bv, out_p):
    nc = tc.nc

    with (
        nc.allow_low_precision(
            reason="fp16 matmul operands; all magnitudes well within fp16 range"
        ),
        tc.tile_pool(name="const", bufs=1) as cpool,
        tc.tile_pool(name="big", bufs=1) as bigpool,
        tc.tile_pool(name="xin", bufs=3) as xin,
        # LAG + 2: av-stream slip beyond LAG (late V units) must not
        # exhaust the P pool, or the sc/exp stream stalls with it
        tc.tile_pool(name="pT", bufs=LAG + 2) as ptpool,
        tc.tile_pool(name="rc", bufs=2) as rcpool,
        tc.tile_pool(name="osb", bufs=3) as osb,
        tc.tile_pool(name="ps", bufs=1, space="PSUM") as psum,
    ):
        # ---- resident weights / biases ----
        xst_sb = cpool.tile([128, 12288], F16, tag="xst")
        # per-mt weight views [128, f, 128]: wX[mt][:, f, :] = W cols
        # mt*128..mt*128+127 for feature chunk f
        wk_mt = [xst_sb[:, 0:1024].rearrange("p (f c) -> p f c", f=FC),
                 xst_sb[:, 10240:11264].rearrange("p (f c) -> p f c", f=FC)]
        wq_mt = [xst_sb[:, 5120:6144].rearrange("p (f c) -> p f c", f=FC),
                 xst_sb[:, 11264:12288].rearrange("p (f c) -> p f c", f=FC)]
        xk0_v = xst_sb[:, 1024:5120].rearrange("p (f s) -> p f s", f=FC)
        xq0_v = xst_sb[:, 6144:10240].rearrange("p (f s) -> p f s", f=FC)
        wpk_sb = cpool.tile([128, 2, FC * HC], F16, tag="wpk")
        wv_sb = wpk_sb[:, 0, :].rearrange("p (f c) -> p f c", f=FC)
        wo_sb = wpk_sb[:, 1, :].rearrange("p (a c) -> p a c", a=2)
        bq_sb = cpool.tile([128, 2], F32, tag="bq")
        bk_sb = cpool.tile([128, 2], F32, tag="bk")
        bv_sb = cpool.tile([128, HC], F32, tag="bv")

        # ---- resident activations ----
        kT_sb = bigpool.tile([128, 2, S], F16, tag="kT")        # K^T (2 m-tiles)
        v_sb = bigpool.tile([128, HPC, KC, 128], F16, tag="v")  # V natural +1s+0pad
        # 2 rotating qT tiles: sc reads of qT[qt] end at step 32qt+31, the
        # aliased writer Q(qt+2) lands no earlier than step 32qt+48
        qT2 = [
            bigpool.tile([128, 2, 512], F16, tag=f"qT{t}", name=f"qT{t}")
            for t in range(2)
        ]
        qT_sb = [qT2[t % 2] for t in range(TT)]
        cT_sb = [
            bigpool.tile([128, 2, 512], F16, tag=f"cT{t}", name=f"cT{t}")
            for t in range(TT)
        ]

        # ---- loads: one engine queue = strict priority order; descriptors
        # fan out across the 16 hardware DMA queues for full bandwidth.
        # Order is the startup critical path: wk, xk0 (f-halves), wq, xq0,
        # then xk1 (halves, JIT for the idx-1..2 filler pumps), the rest.
        def alloc_x(t):
            # 10 pool tiles for the 10 non-startup x tiles (t0's xk/xq live
            # in xst_sb): no aliasing
            return xin.tile([128, FC, 512], F16, tag="xin", bufs=10, name=f"x{t}")

        xk_t = [xk0_v] + [alloc_x(f"k{t}") for t in range(1, TT)]
        xq_t = [xq0_v] + [alloc_x(f"q{t}") for t in range(1, TT)]
        xv_t = [alloc_x(f"v{t}") for t in range(TT)]

        nc.sync.dma_start(xst_sb[:, 0:5120], xst[:, 0:5120])        # wk0|xk0
        nc.sync.dma_start(xst_sb[:, 5120:10240], xst[:, 5120:10240])  # wq0|xq0
        nc.sync.dma_start(xst_sb[:, 10240:12288], xst[:, 10240:12288])  # mt1 w
        nc.sync.dma_start(xk_t[1][:], xk[:, 1])
        nc.sync.dma_start(xk_t[2][:], xk[:, 2])
        nc.sync.dma_start(xk_t[3][:], xk[:, 3])
        nc.sync.dma_start(wpk_sb[:, 0:1], wpk[:, 0:1])          # wv
        for t in range(TT):
            nc.sync.dma_start(xv_t[t][:], xv[:, t])
        nc.sync.dma_start(wpk_sb[:, 1:2], wpk[:, 1:2])          # wo
        for t in range(1, TT):
            nc.sync.dma_start(xq_t[t][:], xq[:, t])
        nc.gpsimd.dma_start(bq_sb[:], bq[:])
        nc.gpsimd.dma_start(bk_sb[:], bk[:])
        nc.gpsimd.dma_start(bv_sb[:], bv[:])

        # ---- warmup: keep the PE activity monitor busy through the initial
        # DMA wait (else projections run at the 1.2 GHz throttled clock) and
        # pull the ~2.7us exp table load off the first real activation.
        # 256-row units quantize the handoff to ~220ns.
        warm_sb = cpool.tile([128, 512], F16, tag="warm")
        nc.vector.memset(warm_sb[:], 0.0)
        warm_ps = psum.tile([128, 512], F32, tag="O", bufs=2, name="warm_ps")
        for _ in range(16):
            nc.tensor.matmul(warm_ps[:, 0:256], lhsT=warm_sb[:, 0:128],
                             rhs=warm_sb[:, 0:256], start=True, stop=True)
        warm_act = cpool.tile([1, 16], F32, tag="warm_act")
        nc.scalar.activation(warm_act[:], warm_sb[0:1, 0:16], AF.Exp, scale=1.0)

        ones_f32 = cpool.tile([128, DK], F32, tag="ones_f32")
        nc.vector.memset(ones_f32[:], 1.0)
        for h in range(HPC):
            nc.vector.tensor_copy(
                v_sb[:, h, :, DK : DK + 1],
                ones_f32[:, 0:KC].rearrange("p (f o) -> p f o", o=1),
            )
            # zero the pad columns so the full-width ctx matmuls (M=128 keeps
            # the PE activity monitor warm) add only zeros
            nc.vector.memset(v_sb[:, h, :, DK + 1 : 128], 0.0)

        # ---- projection emitters (whole; used for the phase-1 criticals) --
        def proj_T_mt(xt, w_mt, b_sb, dst, mt, tag):
            # dst -> AP [128, 512]; computes (x @ W)^T + b for one 128-col
            # chunk; w_mt is the [128, FC, 128] weight view for this chunk
            ps = psum.tile([128, 512], F32, tag=tag, bufs=2, name="psp")
            for f in range(FC):
                nc.tensor.matmul(
                    ps[:],
                    lhsT=w_mt[:, f, :],
                    rhs=xt[:, f, :],
                    start=(f == 0),
                    stop=(f == FC - 1),
                )
            nc.vector.tensor_scalar_add(dst, ps[:], b_sb[:, mt : mt + 1])

        # split variants: 4 units of 2 matmuls (~428ns each on PE)
        def proj_T_mt_units(xt, w_mt, b_sb, dst, mt, tag):
            box = {}

            def unit(f0, box=box):
                if f0 == 0:
                    box["ps"] = psum.tile([128, 512], F32, tag=tag, bufs=2,
                                          name="psp")
                ps = box["ps"]
                for f in (f0, f0 + 1):
                    nc.tensor.matmul(
                        ps[:],
                        lhsT=w_mt[:, f, :],
                        rhs=xt[:, f, :],
                        start=(f == 0),
                        stop=(f == FC - 1),
                    )
                if f0 == FC - 2:
                    nc.vector.tensor_scalar_add(dst, ps[:], b_sb[:, mt : mt + 1])

            return [lambda f0=f0: unit(f0) for f0 in range(0, FC, 2)]

        def v_proj_units(t, j, tag):
            # V in natural layout [tokens, cols]; 2 units of 4 matmuls
            # (4 x 256 moving rows = ~428ns each)
            kt = t * 4 + j
            box = {}

            def unit(f0, box=box):
                if f0 == 0:
                    box["ps"] = psum.tile([128, HC], F32, tag=tag, bufs=2,
                                          name="vps")
                ps = box["ps"]
                for f in range(f0, f0 + 4):
                    nc.tensor.matmul(
                        ps[:],
                        lhsT=xv_t[t][:, f, j * 128 : (j + 1) * 128],
                        rhs=wv_sb[:, f, :],
                        start=(f == 0),
                        stop=(f == FC - 1),
                    )
                if f0 == 4:
                    nc.vector.tensor_add(
                        v_sb[:, :, kt, 0:DK],
                        ps[:].rearrange("p (h c) -> p h c", h=HPC),
                        bv_sb[:].rearrange("p (h c) -> p h c", h=HPC),
                    )

            return [lambda: unit(0), lambda: unit(4)]

        # ---- phase 1 head: K(t0)/Q(0) mt0 halves feed the first score
        # matmul.  Bridge warm units cover the slot1 DMA gap so the PE
        # ramp doesn't reset between the two projections. ----
        proj_T_mt(xk_t[0], wk_mt[0], bk_sb, kT_sb[:, 0, 0:512], 0, "A")
        for _ in range(6):
            nc.tensor.matmul(warm_ps[:, 0:256], lhsT=warm_sb[:, 0:128],
                             rhs=warm_sb[:, 0:256], start=True, stop=True)
        proj_T_mt(xq_t[0], wq_mt[0], bq_sb, qT_sb[0][:, 0, :], 0, "A")

        # ---- filler queues ----
        # urgent: deadline-ordered projection units (K/V/Q), each tagged with
        # the loop idx of its first consumer; before emitting step idx, every
        # unit with deadline <= idx is force-emitted (program order on the
        # in-order PE stream IS correctness -- a late unit would leave the
        # consumer reading uninitialized SBUF).  lazy: out-proj units, no
        # deadline.  Both allocate PSUM only from tag "O".
        urgent = []
        lazy = []

        def pump(n):
            for _ in range(n):
                if urgent:
                    urgent.pop(0)[1]()
                elif lazy:
                    lazy.pop(0)()

        def pump_due(idx):
            while urgent and urgent[0][0] <= idx:
                urgent.pop(0)[1]()

        def kq_units(t, mt, which):
            if which == "k":
                w_mt, b_sb = wk_mt[mt], bk_sb
                dst = kT_sb[:, mt, t * 512 : (t + 1) * 512]
                xt = xk_t[t]
            else:
                w_mt, b_sb = wq_mt[mt], bq_sb
                dst = qT_sb[t][:, mt, :]
                xt = xq_t[t]
            return proj_T_mt_units(xt, w_mt, b_sb, dst, mt, "O")

        def enq(deadline, units):
            urgent.extend((deadline, u) for u in units)

        # deadline-sorted initial queue (deadlines in loop-step index):
        #   K(t)mt0 -> sc 4t ; K(t)mt1 -> sc 16+4t ; Q0mt1 -> sc 16
        #   V(t,j) -> av idx LAG+4t+j ; Q1mt0/mt1 -> sc 32/48
        # K/Q units go BEFORE same-deadline V units: a late K/Q projection
        # hard-stalls the sc/exp stream, while a late V only slips the av
        # stream, which the deep P pool absorbs.
        enq(4, kq_units(1, 0, "k"))
        enq(8, kq_units(2, 0, "k"))
        enq(12, kq_units(3, 0, "k"))
        enq(16, kq_units(0, 1, "k"))
        enq(16, kq_units(0, 1, "q"))
        enq(20, kq_units(1, 1, "k"))
        enq(LAG + 0, v_proj_units(0, 0, "O"))
        enq(LAG + 1, v_proj_units(0, 1, "O"))
        enq(LAG + 2, v_proj_units(0, 2, "O"))
        enq(LAG + 3, v_proj_units(0, 3, "O"))
        enq(24, kq_units(2, 1, "k"))
        enq(LAG + 4, v_proj_units(1, 0, "O"))
        enq(LAG + 5, v_proj_units(1, 1, "O"))
        enq(LAG + 6, v_proj_units(1, 2, "O"))
        enq(LAG + 7, v_proj_units(1, 3, "O"))
        enq(28, kq_units(3, 1, "k"))
        enq(LAG + 8, v_proj_units(2, 0, "O"))
        enq(LAG + 9, v_proj_units(2, 1, "O"))
        enq(LAG + 10, v_proj_units(2, 2, "O"))
        enq(LAG + 11, v_proj_units(2, 3, "O"))
        enq(32, kq_units(1, 0, "q"))
        enq(LAG + 12, v_proj_units(3, 0, "O"))
        enq(LAG + 13, v_proj_units(3, 1, "O"))
        enq(LAG + 14, v_proj_units(3, 2, "O"))
        enq(LAG + 15, v_proj_units(3, 3, "O"))
        enq(48, kq_units(1, 1, "q"))

        def o_proj_units(qt):
            # output projection for token tile qt, computed TRANSPOSED
            # ([out-feature, token]; the host untransposes): wo stationary,
            # cT moving.  Partial (host sums head groups).  2 units per
            # 128-outcol chunk, interleaved mms/stores so each oc's matmuls
            # run a few sites ahead of its store.
            units = []
            for oc in range(8):
                box = {}

                def mms(oc=oc, box=box):
                    o_ps = psum.tile([128, 512], F32, tag="O", bufs=2, name="ops")
                    box["ps"] = o_ps
                    for c2 in range(2):
                        nc.tensor.matmul(
                            o_ps[:],
                            lhsT=wo_sb[:, c2, oc * 128 : (oc + 1) * 128],
                            rhs=cT_sb[qt][:, c2, :],
                            start=(c2 == 0),
                            stop=(c2 == 1),
                        )

                def store(oc=oc, box=box):
                    ob = osb.tile([128, 512], F16, tag="ob")
                    nc.vector.tensor_copy(ob[:], box["ps"][:])
                    # alternate store queues so back-to-back stores overlap
                    q = nc.gpsimd if oc % 2 == 0 else nc.sync
                    q.dma_start(
                        out_p[oc * 128 : (oc + 1) * 128,
                              qt * 512 : (qt + 1) * 512],
                        ob[:],
                    )

                units += [mms, store]
            mm, st = units[0::2], units[1::2]
            return [mm[0], mm[1], st[0], mm[2], st[1], mm[3], st[2], mm[4],
                    st[3], mm[5], st[4], mm[6], st[5], mm[7], st[6], st[7]]

        def enqueue_boundary(qt):
            # Q1 is already in the initial queue (its deadline falls inside
            # the qt0 projection crunch); Q2/Q3 enqueue at boundaries when
            # urgent has drained
            if 0 < qt < TT - 1:
                for mt in range(2):
                    enq(32 * (qt + 1) + 16 * mt, kq_units(qt + 1, mt, "q"))
            if qt > 0:
                lazy.extend(o_proj_units(qt - 1))

        # ---- attention: one global software pipeline over 128 steps ----
        steps = [(qt, hp, kc) for qt in range(TT) for hp in range(2)
                 for kc in range(KC)]
        Cs = {}
        Ps = {}

        # Schraudolph fp16 exp on DVE for a sparse subset of steps: relieves
        # ScalarE when it paces.  i16 = s*(0.125*1024*log2 e) + (15<<10) - 48
        # bit-cast to fp16 gives exp(s*0.125) to ~3% (host-validated: relmax
        # vs reference stays ~1.3e-2, under the 2e-2 gate).  Steps in the
        # qt0/qt1 projection crunch and the qt3 store window stay on ScalarE
        # so DVE queue delays can't stall the P handoff.
        SCH_SCALE = 0.125 * 1024.0 * 1.4426950408889634
        SCH_BIAS = 15.0 * 1024.0 - 48.0
        I16 = mybir.dt.int16

        def sc_exp(qt, hp, kc, offload=False):
            A = psum.tile([128, 2, 512], F32, tag="A", bufs=2, name="A")
            for i in range(2):
                p0 = i * 64
                # the adjacent row-packed score matmuls (rows 0:64 / 64:128
                # via lhsT base_partition) run concurrently in disjoint PE
                # row groups
                nc.tensor.matmul(
                    A[:, i, :],
                    lhsT=kT_sb[p0 : p0 + 64, hp, kc * 128 : (kc + 1) * 128],
                    rhs=qT_sb[qt][p0 : p0 + 64, hp, :],
                    start=True,
                    stop=True,
                )
            if offload:
                P = ptpool.tile([128, 2, 512], I16, tag="pT", name="P")
                nc.vector.tensor_scalar(
                    P[:].rearrange("p a b -> p (a b)"),
                    A[:].rearrange("p a b -> p (a b)"),
                    SCH_SCALE,
                    SCH_BIAS,
                    mybir.AluOpType.mult,
                    mybir.AluOpType.add,
                )
            else:
                P = ptpool.tile([128, 2, 512], F16, tag="pT", name="P")
                nc.scalar.activation(
                    P[:].rearrange("p a b -> p (a b)"),
                    A[:].rearrange("p a b -> p (a b)"),
                    AF.Exp,
                    scale=0.125,
                )
            Ps[(qt, hp, kc)] = (P, offload)

        def av(qt, hp, kc):
            if kc == 0:
                Cs[(qt, hp)] = psum.tile([128, 2, 512], F32, tag="C", bufs=1,
                                         name="C")
            C = Cs[(qt, hp)]
            P, is_i16 = Ps.pop((qt, hp, kc))
            for i in range(2):
                rhs = P[:, i, :]
                if is_i16:
                    rhs = rhs.bitcast(F16)
                nc.tensor.matmul(
                    C[:, i, :],
                    lhsT=v_sb[:, 2 * hp + i, kc, :],
                    rhs=rhs,
                    start=(kc == 0),
                    stop=(kc == KC - 1),
                )

        def normalize(qt, hp):
            C = Cs.pop((qt, hp))
            # copy ctx+denominator out of PSUM first: the C accumulator is
            # released after these two copies, so the next head-pair's ctx
            # matmuls only wait ~1.3us; the rest runs off the critical path.
            # For the very last head-pair nothing waits on C, so skip the
            # staging copies and shorten the end-of-kernel serial chain.
            last = qt == TT - 1 and hp == 1
            cUs = []
            for i in range(2):
                if last:
                    cUs.append(C[:, i, :])
                    continue
                # two per-head staging copies: av(i) of the next head-pair
                # only waits on copy(i) (region-level deps), and the copies
                # MUST stay on DVE -- a ScalarE activation-Copy here forces
                # an ACT-table reload before the next Exp (+1283ns each)
                cU = rcpool.tile([DK + 1, 512], F32, tag="cU", name="cU")
                nc.vector.tensor_copy(cU[:], C[0 : DK + 1, i, :])
                cUs.append(cU)
            for i, cU in enumerate(cUs):
                p0 = i * 64
                # custom-DVE ops ignore the input base partition, so the
                # denominator row must be relocated to p0 by a builtin copy
                # custom-DVE ops (reciprocal_approx_fast, partition_broadcast)
                # ignore the input base partition, so drow/rc must be tiles
                # whose partition range starts at p0; bufs=1 on the
                # off-critical-path scratch keeps SBUF pressure down
                drow = rcpool.tile([1, 512], F32, tag="drow", bufs=1)
                nc.vector.tensor_copy(drow[:], cU[DK : DK + 1, :])
                rc = rcpool.tile([1, 512], F32, tag="rc", bufs=1)
                nc.vector.reciprocal_approx_fast(rc[:], drow[:])
                Sb = rcpool.tile([DK, 512], F32, tag="Sb", bufs=1)
                nc.gpsimd.partition_broadcast(Sb[:], rc[:])
                nc.vector.tensor_mul(
                    cT_sb[qt][p0 : p0 + 64, hp, :], cU[0:DK, :], Sb[:]
                )

        for idx in range(len(steps) + LAG):
            pump_due(idx)
            if idx < len(steps):
                qt, hp, kc = steps[idx]
                # NOTE: Schraudolph-on-DVE offload measured NET-NEGATIVE here:
                # the offloaded step's A-PSUM release moves to the DVE queue,
                # which stalls sc(i+2) by ~0.8-1us per offloaded step
                sc_exp(qt, hp, kc, offload=False)
                if 1 <= idx < LAG:
                    # no ctx matmuls yet: ~900ns of PE slack per step
                    pump(2)
            if idx >= LAG:
                qt, hp, kc = steps[idx - LAG]
                if hp == 0 and kc == 0:
                    enqueue_boundary(qt)
                # av directly after sc keeps the PE stream back-to-back (one
                # dependency break per step instead of two -- each break
                # exposes the ~165ns pipeline-fill latency); the pumped unit
                # goes last, where its possible DMA wait merges with the next
                # step's unavoidable A-buffer wait
                av(qt, hp, kc)
                pump(1)
                if kc == KC - 1:
                    normalize(qt, hp)

        # tail: flush leftovers, then the last token tile's output projection,
        # split by contraction half.  The c2=0 matmuls depend only on the hp0
        # normalize (ready well before the end), so they execute during the
        # final hp1 normalize chain; only the c2=1 accumulation and the
        # stores remain on the serial tail.  A-tag PSUM is free after the
        # last scores, giving 6 resident partial slots (4 A halves + 2 O).
        pump(len(urgent) + len(lazy))
        assert not urgent and not lazy
        qt = TT - 1
        ta = [psum.tile([128, 2, 512], F32, tag="A", bufs=2, name="ota")
              for _ in range(2)]
        slots = [t[:, i, :] for t in ta for i in range(2)]
        slots += [psum.tile([128, 512], F32, tag="O", bufs=2, name="otb")
                  for _ in range(2)]
        for oc in range(6):
            nc.tensor.matmul(
                slots[oc],
                lhsT=wo_sb[:, 0, oc * 128 : (oc + 1) * 128],
                rhs=cT_sb[qt][:, 0, :],
                start=True, stop=False,
            )

        # keep-warm matmuls bridge the ~3.5us final-normalize wait so the
        # c2=1 pass and stores run at the 2.4 GHz clock; they accumulate 0*0
        # into oc0's open group, preserving its c2=0 partial
        for _ in range(16):
            nc.tensor.matmul(slots[0], lhsT=warm_sb[:, 0:128],
                             rhs=warm_sb[:], start=False, stop=False)

        def o_store(oc, ps):
            ob = osb.tile([128, 512], F16, tag="ob")
            # alternate the PSUM->SBUF cast between DVE and ScalarE (idle
            # after the last exp) so the final 8 stores aren't serialized
            # on one engine
            if oc % 2 == 0:
                nc.vector.tensor_copy(ob[:], ps)
            else:
                nc.scalar.copy(ob[:], ps)
            q = nc.gpsimd if oc % 2 == 0 else nc.sync
            q.dma_start(
                out_p[oc * 128 : (oc + 1) * 128, qt * 512 : (qt + 1) * 512],
                ob[:],
            )

        for oc in range(6):
            nc.tensor.matmul(
                slots[oc],
                lhsT=wo_sb[:, 1, oc * 128 : (oc + 1) * 128],
                rhs=cT_sb[qt][:, 1, :],
                start=False, stop=True,
            )
            o_store(oc, slots[oc])
        # ocs 6/7 go into the C-tag banks, which the final normalize has just
        # released -- the O-tag rotation at this point is serialized behind
        # the leftover store casts (~4us wait measured)
        c_ps = psum.tile([128, 2, 512], F32, tag="C", bufs=1, name="otc")
        for j, oc in enumerate((6, 7)):
            for c2 in range(2):
                nc.tensor.matmul(
                    c_ps[:, j, :],
                    lhsT=wo_sb[:, c2, oc * 128 : (oc + 1) * 128],
                    rhs=cT_sb[qt][:, c2, :],
                    start=(c2 == 0), stop=(c2 == 1),
                )
            o_store(oc, c_ps[:, j, :])


def _tile_x(xb):
    # [D, S] -> [128, TT, FC, 512] with X[p, t, f, s] = x[f*128 + p, t*512 + s]
    # so each 512-token tile is one fully contiguous 8KB-per-partition DMA
    return np.ascontiguousarray(
        xb.reshape(FC, 128, TT, 512).transpose(1, 2, 0, 3).astype(np.float16)
    )


def _tile_w(w):
    # [D, C] -> [128, FC, C] with W[p, f, c] = w[f*128 + p, c]
    c = w.shape[1]
    return np.ascontiguousarray(
        w.reshape(FC, 128, c).transpose(1, 0, 2).astype(np.float16)
    )


def _tile_wo(w):
    # [HC, D] -> [128, 2, D]
    return np.ascontiguousarray(
        w.reshape(2, 128, D).transpose(1, 0, 2).astype(np.float16)
    )


def _shard_inputs(query, key_in, value, Wq, bq, Wk, bk, Wv, bv, Wo, bo):
    q = np.asarray(query, dtype=np.float32)
    k = np.asarray(key_in, dtype=np.float32)
    v = np.asarray(value, dtype=np.float32)
    Wq, Wk, Wv, Wo = (np.asarray(a, np.float32) for a in (Wq, Wk, Wv, Wo))
    bq, bk, bv = (np.asarray(a, np.float32) for a in (bq, bk, bv))

    # per-batch tiled fp16 activations, shared across the 4 head groups
    xT = {b: tuple(_tile_x(x[b].T) for x in (q, k, v)) for b in range(2)}

    in_maps = []
    for core in range(8):
        b, g = divmod(core, 4)
        sl = slice(g * HC, (g + 1) * HC)
        xq_t, xk_t, xv_t = xT[b]
        wk_p = _tile_w(Wk[:, sl])   # [128, FC, 256]
        wq_p = _tile_w(Wq[:, sl])
        in_maps.append(
            {
                "xstart": np.ascontiguousarray(np.concatenate([
                    wk_p[:, :, 0:128].reshape(128, 1024),
                    xk_t[:, 0].reshape(128, 4096),
                    wq_p[:, :, 0:128].reshape(128, 1024),
                    xq_t[:, 0].reshape(128, 4096),
                    wk_p[:, :, 128:256].reshape(128, 1024),
                    wq_p[:, :, 128:256].reshape(128, 1024),
                ], axis=1)),
                "xq_t": xq_t,
                "xk_t": xk_t,
                "xv_t": xv_t,
                "wpack": np.ascontiguousarray(np.stack([
                    _tile_w(Wv[:, sl]).reshape(128, FC * HC),
                    _tile_wo(Wo[sl, :]).reshape(128, FC * HC),
                ], axis=1)),
                "bq2": np.ascontiguousarray(bq[sl].reshape(2, 128).T),
                "bk2": np.ascontiguousarray(bk[sl].reshape(2, 128).T),
                "bv_bc": np.ascontiguousarray(
                    np.broadcast_to(bv[sl], (128, HC))
                ),
            }
        )
    return in_maps


def kernel(query=None, key_in=None, value=None, Wq=None, bq=None, Wk=None,
           bk=None, Wv=None, bv=None, Wo=None, bo=None, key=None, **_unused):
    global LAST_RESULTS, _NC_CACHE
    if key_in is None:
        key_in = key
    if _NC_CACHE is None:
        _NC_CACHE = build_nc()
    nc = _NC_CACHE

    in_maps = _shard_inputs(query, key_in, value, Wq, bq, Wk, bk, Wv, bv, Wo, bo)
    trace = bool(os.environ.get("BASS_TRACE"))
    res = run_bass_kernel_spmd(nc, in_maps, core_ids=list(range(8)), trace=trace)
    LAST_RESULTS = res

    bo = np.asarray(bo, np.float32)
    out = np.empty((2, S, D), dtype=np.float32)
    for b in range(2):
        acc = res.results[4 * b]["out_p"].astype(np.float32)
        for g in range(1, 4):
            acc = acc + res.results[4 * b + g]["out_p"].astype(np.float32)
        out[b] = acc.T + bo
    return out
